# revision 9
# baseline (speedup 1.0000x reference)
"""Distributed GQA attention (RoPE + causal mask + o_proj) on 8 TRN2 NeuronCores.

Sharding: 8-way tensor parallel over heads. Core c handles q heads
[4c, 4c+4) and kv head c (the matching GQA group) for BOTH batches.

v3: phase-interleaved emission. The attention stream (ACT-bound: one exp
per k-tile) is the backbone; projection and o_proj matmul chains are
split into ~1us "filler units" pumped between attention k-tile groups so
the in-order PE never idles while the ACT engine works through the exps,
and vice versa.  Other changes vs the phase-serial baseline:
  RoPE swap-halves via SBUF->SBUF partition-block DMAs (was: a PE
  permutation matmul per 512-row slice through an extra PSUM pool)
  v^T -> v transposes via the XBAR transpose DMA (was: PE transposes)
  one PSUM "chain" pool (2 banks) shared by q/kv/o_proj accumulation
  chains; scores double-buffered (4 banks); attn accumulator (2 banks)
  wo prefetch deferred into the attention(b0) window; o_proj(b0) units
  pump during attention(b1)
"""

import os
import sys

for _p in ("/opt/trn_rl_repo", "/root/.axon_site/_ro/trn_rl_repo"):
    if os.path.isdir(_p) and _p not in sys.path:
        sys.path.append(_p)

from collections import deque

import numpy as np
import ml_dtypes

import concourse.bass as bass
import concourse.bacc as bacc
import concourse.tile as tile
import concourse.mybir as mybir
from concourse import bass_utils

FP32 = mybir.dt.float32
BF16 = mybir.dt.bfloat16
AF = mybir.ActivationFunctionType
ALU = mybir.AluOpType
PSUM = bass.MemorySpace.PSUM
NPBF16 = ml_dtypes.bfloat16

B = 2
D = 2048
S = 2048
HD = 64
HD1 = HD + 1
VBS = 80     # v-block stride: 65 used cols padded to a 16-element-aligned stride
             # (the XBAR transpose DMA silently drops writes at dst column
             # offsets that are not multiples of 16 elements)
N_HEADS = 32
N_KV = 8
NCORES = 8
HQ = N_HEADS // NCORES   # 4 local q heads
QCOLS = HQ * HD          # 256
NDC = D // 128           # 16 contraction chunks
NPAIR = HQ // 2          # 2 head pairs
THETA = 10000.0


def build_graph(causal: bool, s: int = S):
    """Build + compile the per-core SPMD graph. Identical on all 8 cores."""
    bs = B * s               # flattened rows
    nqb = s // 512           # q blocks per batch
    nkt = s // 128           # k tiles per batch
    rows_h = s // NCORES     # output rows per core per batch
    rt_m = min(128, rows_h)  # o_proj row-tile height
    nrt_h = rows_h // rt_m   # o_proj row tiles per batch half

    nc = bacc.Bacc("TRN2", target_bir_lowering=False, debug=False,
                   enable_asserts=True, num_devices=NCORES)

    xT_h = nc.dram_tensor("xT", [D, bs], BF16, kind="ExternalInput")
    wq_h = nc.dram_tensor("wq", [D, QCOLS], BF16, kind="ExternalInput")
    wkv_h = nc.dram_tensor("wkv", [D, 2 * HD], BF16, kind="ExternalInput")
    wo_h = nc.dram_tensor("wo", [D, D], BF16, kind="ExternalInput")
    c4_h = nc.dram_tensor("c4", [128, s], BF16, kind="ExternalInput")
    s4_h = nc.dram_tensor("s4", [128, s], BF16, kind="ExternalInput")
    if causal:
        md_h = nc.dram_tensor("mdiag", [128, 128], BF16, kind="ExternalInput")
    else:
        mT_h = nc.dram_tensor("maskT", [s, s], FP32, kind="ExternalInput")
    out_h = nc.dram_tensor("out", [B * rows_h, D], FP32, kind="ExternalOutput")

    with tile.TileContext(nc) as tc:
        with tc.tile_pool(name="persist", bufs=1) as pp, \
             tc.tile_pool(name="dram", bufs=1, space="DRAM") as dramp:

            # ---- persistent activations ----
            qT = [pp.tile([128, bs], BF16, tag=f"qT{i}", name=f"qT{i}")
                  for i in range(NPAIR)]
            kTrep = [pp.tile([128, s], BF16, tag=f"kTr{i}", name=f"kTrep{i}")
                     for i in range(B)]
            # v per batch: nkt blocks of [128, 65]; v at cols 0:HD of each
            # block, ones col at HD (softmax denominator row)
            vbt = [pp.tile([128, nkt * VBS], BF16, tag=f"vb{b}",
                           name=f"vbt{b}") for b in range(B)]
            # attn^T, one [64, s] tile per local head, reused across batches
            attnT = [pp.tile([64, s], BF16, tag=f"aT{i}", name=f"attnT{i}")
                     for i in range(HQ)]
            c4 = pp.tile([128, s], BF16, tag="c4", name="c4t")
            s4 = pp.tile([128, s], BF16, tag="s4", name="s4t")
            if causal:
                md = pp.tile([128, 128], BF16, tag="mdb", name="mdb")

            bnc_in = [[dramp.tile([NCORES, 2 * HD, rows_h], BF16,
                                  tag=f"bin{b}_{hp}", name=f"bounce_in{b}_{hp}")
                       for hp in range(NPAIR)] for b in range(B)]
            bnc_out = [[dramp.tile([NCORES, 2 * HD, rows_h], BF16,
                                   tag=f"bout{b}_{hp}", name=f"bounce_out{b}_{hp}")
                        for hp in range(NPAIR)] for b in range(B)]

            with tc.tile_pool(name="xpool", bufs=16) as xp, \
                 tc.tile_pool(name="ropeq", bufs=2) as rpq, \
                 tc.tile_pool(name="ropek", bufs=1) as rpk, \
                 tc.tile_pool(name="chain", bufs=2, space=PSUM) as chp, \
                 tc.tile_pool(name="sc_p", bufs=2, space=PSUM) as psc, \
                 tc.tile_pool(name="at_p", bufs=1, space=PSUM) as pat, \
                 tc.tile_pool(name="probs", bufs=4) as prp, \
                 tc.tile_pool(name="maskq", bufs=4) as mqp, \
                 tc.tile_pool(name="norm", bufs=1) as nrm, \
                 tc.tile_pool(name="att2", bufs=1) as a2p, \
                 tc.tile_pool(name="yout", bufs=2) as yop:

                # wres opened last so it sits on top of the pool stack
                # and can be released mid-emission (LIFO requirement)
                wrp_ctx = tc.tile_pool(name="wres", bufs=1)
                wrp = wrp_ctx.__enter__()

                # ================= startup DMAs ========================
                # consumption-ordered: first matmul needs wq dc0 (scalar)
                # + x tile dc0 (sync).
                nxp = bs // 1024      # xT tiles, each covering 2 row slices
                xtiles = [[None] * NDC for _ in range(nxp)]
                wq_all = wrp.tile([128, NDC * QCOLS], BF16, tag="wqa",
                                  name="wq_all")
                wqb = [wq_all[:, dc * QCOLS:(dc + 1) * QCOLS]
                       for dc in range(NDC)]
                for dc in range(NDC):
                    nc.scalar.dma_start(wqb[dc],
                                        wq_h[dc * 128:(dc + 1) * 128, :])
                    xt = xp.tile([128, 1024], BF16, tag="xtb",
                                 name=f"xtb0_{dc}")
                    eng = nc.sync if dc % 2 == 0 else nc.scalar
                    eng.dma_start(xt[:, :],
                                  xT_h[dc * 128:(dc + 1) * 128, 0:1024])
                    xtiles[0][dc] = xt

                wkv_all = wrp.tile([128, NDC * 2 * HD], BF16, tag="wkva",
                                   name="wkv_all")
                for g in range(4):
                    dstk = wkv_all[:, g * 8 * HD:(g + 1) * 8 * HD].rearrange(
                        "p (c q) -> p c q", c=4)
                    srck = wkv_h[g * 512:(g + 1) * 512, :].rearrange(
                        "(c p) q -> p c q", p=128)
                    nc.gpsimd.dma_start(dstk, srck)
                wkvb = [wkv_all[:, dc * 2 * HD:(dc + 1) * 2 * HD]
                        for dc in range(NDC)]
                nc.gpsimd.dma_start(c4[:, :], c4_h[:, :])
                nc.gpsimd.dma_start(s4[:, :], s4_h[:, :])
                if causal:
                    nc.gpsimd.dma_start(md[:, :], md_h[:, :])

                # x tiles for b0's second half on sync/scalar (startup,
                # ACT still idle)
                for dc in range(NDC):
                    xt = xp.tile([128, 1024], BF16, tag="xtb",
                                 name=f"xtb1_{dc}")
                    eng = nc.sync if dc % 2 == 0 else nc.scalar
                    eng.dma_start(xt[:, :],
                                  xT_h[dc * 128:(dc + 1) * 128, 1024:2048])
                    xtiles[1][dc] = xt

                # ================= filler machinery =====================
                fill = deque()
                pending = []          # (due_pump_no, unit)
                pump_no = [0]

                def pump(n):
                    pump_no[0] += 1
                    while pending and pending[0][0] <= pump_no[0]:
                        fill.append(pending.pop(0)[1])
                    k = 0
                    while fill and k < n:
                        fill.popleft()()
                        k += 1

                def schedule(units, delay, stagger=1):
                    due = pump_no[0] + delay
                    for i, u in enumerate(units):
                        pending.append((due + i * stagger, u))
                    pending.sort(key=lambda t: t[0])

                # ================= projection slice units ================
                def swap_dma(dst_t, src_t, npart):
                    # dst[a*64+b*32+p] = src[a*64+(1-b)*32+p] (swap 32-halves
                    # inside each 64-block); SBUF->SBUF on the sync queue
                    for a in range(npart // 64):
                        o = a * 64
                        nc.sync.dma_start(dst_t[o:o + 32, :],
                                          src_t[o + 32:o + 64, :])
                        nc.sync.dma_start(dst_t[o + 32:o + 64, :],
                                          src_t[o:o + 32, :])

                def rope(src_ps, swp_sb, dst_ap, sl0, npart, nm):
                    """dst = src .* c4 + swap(src) .* s4."""
                    pool = rpq if npart == 128 else rpk
                    cs = c4[0:npart, sl0:sl0 + 512]
                    sn = s4[0:npart, sl0:sl0 + 512]
                    m1 = pool.tile([npart, 512], BF16, tag=f"m1_{npart}",
                                   name=f"m1{nm}")
                    nc.vector.tensor_mul(m1[:, :], src_ps, cs)
                    m2 = pool.tile([npart, 512], BF16, tag=f"m2_{npart}",
                                   name=f"m2{nm}")
                    nc.vector.tensor_mul(m2[:, :], swp_sb, sn)
                    nc.vector.tensor_add(dst_ap, m1[:, :], m2[:, :])

                def proj_slice_units(xi, sub):
                    """Return filler units (closures) for one 512-row slice."""
                    rs = xi * 2 + sub
                    r0 = rs * 512          # flattened row offset
                    b = r0 // s            # batch of this slice
                    sl0 = r0 - b * s       # seq offset within batch
                    xsl = slice(sub * 512, sub * 512 + 512)
                    st = {}
                    units = []

                    def q_chain_mm(hp, dc0):
                        def u():
                            if dc0 == 0:
                                st[hp] = chp.tile([128, 512], FP32,
                                                  tag="chain",
                                                  name=f"qps{rs}_{hp}")
                            qps = st[hp]
                            for dc in range(dc0, dc0 + 4):
                                nc.tensor.matmul(
                                    qps[:, :],
                                    wqb[dc][:, hp * 128:(hp + 1) * 128],
                                    xtiles[xi][dc][:, xsl],
                                    start=(dc == 0), stop=(dc == NDC - 1))
                            if dc0 == 12:
                                qsb = rpq.tile([128, 512], BF16, tag="qsb",
                                              name=f"qsb{rs}_{hp}")
                                nc.scalar.copy(qsb[:, :], qps[:, :])
                                qsw = rpq.tile([128, 512], BF16, tag="qsw",
                                              name=f"qsw{rs}_{hp}")
                                swap_dma(qsw, qsb, 128)
                                rope(qps[:, :], qsw[:, :],
                                     qT[hp][:, r0:r0 + 512], sl0, 128,
                                     f"q{rs}_{hp}")
                        return u

                    def kv_chain_mm(dc0):
                        def u():
                            if dc0 == 0:
                                st["kv"] = chp.tile([128, 512], FP32,
                                                    tag="chain",
                                                    name=f"kvps{rs}")
                            kvps = st["kv"]
                            for dc in range(dc0, dc0 + 4):
                                nc.tensor.matmul(kvps[:, :], wkvb[dc][:, :],
                                                 xtiles[xi][dc][:, xsl],
                                                 start=(dc == 0),
                                                 stop=(dc == NDC - 1))
                            if dc0 == 12:
                                # k: copy, swap, rope, write to both kTrep
                                # partition halves
                                ksb = rpk.tile([64, 512], BF16, tag="ksb",
                                              name=f"ksb{rs}")
                                nc.scalar.copy(ksb[:, :], kvps[0:64, :])
                                ksw = rpk.tile([64, 512], BF16, tag="ksw",
                                              name=f"ksw{rs}")
                                swap_dma(ksw, ksb, 64)
                                ktmp = rpk.tile([64, 512], BF16, tag="ktmp",
                                               name=f"ktmp{rs}")
                                rope(kvps[0:64, :], ksw[:, :], ktmp[:, :],
                                     sl0, 64, f"k{rs}")
                                nc.sync.dma_start(
                                    kTrep[b][0:64, sl0:sl0 + 512], ktmp[:, :])
                                nc.sync.dma_start(
                                    kTrep[b][64:128, sl0:sl0 + 512],
                                    ktmp[:, :])
                                # v: evacuate to SBUF, XBAR-transpose into
                                # the per-kt stationary blocks, set ones col
                                vts = rpk.tile([128, 512], BF16, tag="vts",
                                              name=f"vts{rs}")
                                nc.vector.tensor_copy(vts[64:128, :],
                                                      kvps[64:128, :])
                                for rb in range(4):
                                    kt = sl0 // 128 + rb
                                    o = kt * VBS
                                    nc.sync.dma_start_transpose(
                                        vbt[b][:, o:o + HD],
                                        vts[64:128, rb * 128:(rb + 1) * 128])
                                    nc.vector.memset(
                                        vbt[b][:, o + HD:o + HD + 1], 1.0)
                        return u

                    for hp in range(NPAIR):
                        for dc0 in (0, 4, 8, 12):
                            units.append(q_chain_mm(hp, dc0))
                    for dc0 in (0, 4, 8, 12):
                        units.append(kv_chain_mm(dc0))
                    return units

                def load_xi_units(xi):
                    # b1 x tiles all on sync: the scalar/ACT queue must stay
                    # free for exp, and gpsimd head-blocks on the per-qb
                    # partition_broadcast waits
                    def mk(dcs):
                        def u():
                            x0 = xi * 1024
                            for dc in dcs:
                                xt = xp.tile([128, 1024], BF16, tag="xtb",
                                             name=f"xtb{xi}_{dc}")
                                nc.sync.dma_start(
                                    xt[:, :],
                                    xT_h[dc * 128:(dc + 1) * 128,
                                         x0:x0 + 1024])
                                xtiles[xi][dc] = xt
                        return u
                    return [mk(range(0, 8)), mk(range(8, 16))]

                # ================= wo prefetch units =====================
                # wob (64K/partition) opens only after wres releases, both
                # as fill closures so emission order frees the space first
                wo_state = {}
                wot = []

                def release_wres():
                    wrp_ctx.__exit__(None, None, None)
                    wo_state["ctx"] = tc.tile_pool(name="wob", bufs=1)
                    wo_state["pool"] = wo_state["ctx"].__enter__()

                def wo_units():
                    def mk(chs):
                        def u():
                            for ch in chs:
                                wt = wo_state["pool"].tile(
                                    [128, D], BF16, tag=f"wob{ch}",
                                    name=f"wob{ch}")
                                eng = nc.sync if ch % 2 == 0 else nc.gpsimd
                                eng.dma_start(wt[:, :],
                                              wo_h[ch * 128:(ch + 1) * 128, :])
                                wot.append(wt)
                        return u
                    return [mk(range(0, 4)), mk(range(4, 8)),
                            mk(range(8, 12)), mk(range(12, 16))]

                # ================= attention =============================
                def attn_block(b, hp, qb):
                    q0 = qb * 512            # seq offset within batch
                    g0 = b * s + q0          # flattened offset
                    kt_end = 4 * (qb + 1) if causal else nkt
                    mts = {}
                    # packed accumulator: par0 cols 0:512, par1 rest
                    acc = pat.tile([HD1, 1024], FP32, tag="acc",
                                   name=f"acc{b}_{qb}_{hp}")
                    # two-deep software pipeline as in the baseline:
                    # clean/full @v of kt-1 and the masked strip of kt-2
                    # are emitted after scores(kt)
                    pend_cl = None   # (kt, qs, pb, diag)
                    pend_st = None   # (kt, qs, pb)

                    def av_clean(kt_, qs_, pb_, diag_, sp=False):
                        stt = kt_ == 0
                        for par in range(2):
                            c0 = par * 512
                            vsl = vbt[b][:, kt_ * VBS:kt_ * VBS + HD1]
                            if diag_ and not stt:
                                if qs_ < 384:
                                    nc.tensor.matmul(
                                        acc[:, c0 + qs_ + 128:c0 + 512],
                                        vsl,
                                        pb_[:, c0 + qs_ + 128:c0 + 512],
                                        start=False, stop=False)
                            else:
                                nc.tensor.matmul(
                                    acc[:, c0 + qs_:c0 + 512],
                                    vsl,
                                    pb_[:, c0 + qs_:c0 + 512],
                                    start=stt, stop=(sp and par == 1))

                    def av_strip(kt_, qs_, pb_, sp):
                        for par in range(2):
                            c0 = par * 512
                            nc.tensor.matmul(
                                acc[:, c0 + qs_:c0 + qs_ + 128],
                                vbt[b][:, kt_ * VBS:kt_ * VBS + HD1],
                                pb_[:, c0 + qs_:c0 + qs_ + 128],
                                start=False,
                                stop=(sp and par == 1))

                    for kt in range(kt_end):
                        k0 = kt * 128
                        diag = causal and kt >= 4 * qb
                        qs = (k0 - q0) if diag else 0
                        if not causal:
                            mt = mqp.tile([128, 512], FP32, tag="mq",
                                          name=f"mq{b}_{hp}_{qb}_{kt}")
                            nc.sync.dma_start(
                                mt[:, :],
                                mT_h[k0:k0 + 128, q0:q0 + 512])
                            mts[kt] = mt
                        sc = psc.tile([128, 1024], FP32, tag="sc",
                                      name=f"sc{b}_{qb}_{hp}_{kt}")
                        for par in range(2):
                            pr = par * 64
                            c0 = par * 512
                            nc.tensor.matmul(
                                sc[:, c0 + qs:c0 + 512],
                                kTrep[b][pr:pr + 64, k0:k0 + 128],
                                qT[hp][pr:pr + 64, g0 + qs:g0 + 512],
                                start=True, stop=True)
                            if not causal:
                                nc.vector.tensor_add(
                                    sc[:, c0:c0 + 512],
                                    sc[:, c0:c0 + 512],
                                    mts[kt][:, :])
                        if pend_st is not None:
                            av_strip(*pend_st, sp=False)
                            pend_st = None
                        if pend_cl is not None:
                            av_clean(*pend_cl)
                            if pend_cl[3] and pend_cl[0] != 0:
                                pend_st = pend_cl[:3]
                            pend_cl = None
                        pb = prp.tile([128, 1024], BF16, tag="pb",
                                      name=f"pb{b}_{qb}_{hp}_{kt}")
                        if qs > 0 and qb == 0:
                            # PSUM slot not fully written yet: exp each
                            # valid range separately so stale garbage
                            # can't produce inf
                            nc.scalar.activation(pb[:, qs:512],
                                                 sc[:, qs:512], AF.Exp)
                            nc.scalar.activation(pb[:, 512 + qs:1024],
                                                 sc[:, 512 + qs:1024],
                                                 AF.Exp)
                        else:
                            nc.scalar.activation(pb[:, qs:1024],
                                                 sc[:, qs:1024], AF.Exp)
                        if diag:
                            msl = md[:, 0:128]
                            for par in range(2):
                                c0 = par * 512
                                nc.vector.tensor_mul(
                                    pb[:, c0 + qs:c0 + qs + 128],
                                    pb[:, c0 + qs:c0 + qs + 128], msl)
                        pend_cl = (kt, qs, pb, diag)
                        pump(1)
                    # flush
                    if pend_st is not None:
                        av_strip(*pend_st, sp=False)
                    has_strip = pend_cl[3] and pend_cl[0] != 0
                    av_clean(*pend_cl, sp=not has_strip)
                    if has_strip:
                        av_strip(*pend_cl[:3], sp=True)
                    # normalization: PSUM -> SBUF, row-hop the denominator
                    # row, reciprocal, partition-broadcast, scale
                    a_sb = nrm.tile([HD1, 1024], FP32, tag="asb",
                                    name=f"asb{b}_{qb}_{hp}")
                    nc.vector.tensor_copy(a_sb[:, :], acc[:, :])
                    # row-hop on the scalar queue: sync head-blocks for
                    # ~25us on the att2 loads waiting out the collective
                    rc = nrm.tile([1, 1024], FP32, tag="rc",
                                  name=f"rc{b}_{qb}_{hp}")
                    nc.scalar.dma_start(rc[0:1, :], a_sb[HD:HD + 1, :])
                    rcr = nrm.tile([1, 1024], FP32, tag="rcr",
                                   name=f"rcr{b}_{qb}_{hp}")
                    nc.vector.reciprocal_approx_fast(rcr[0:1, :], rc[0:1, :])
                    rcb = nrm.tile([64, 1024], FP32, tag="rcb",
                                   name=f"rcb{b}_{qb}_{hp}")
                    nc.gpsimd.partition_broadcast(rcb[:, :], rcr[0:1, :])
                    for par in range(2):
                        head = hp * 2 + par
                        c0 = par * 512
                        nc.vector.tensor_mul(
                            attnT[head][:, q0:q0 + 512],
                            a_sb[0:HD, c0:c0 + 512],
                            rcb[:, c0:c0 + 512])
                        # per-qb bounce-in slice: keeps the sync queue from
                        # head-blocking on the whole head at the A2A point
                        src = attnT[head][:, q0:q0 + 512].rearrange(
                            "p (j c) -> p j c", j=2)
                        dst = bnc_in[b][hp][2 * qb:2 * qb + 2,
                                            par * 64:(par + 1) * 64,
                                            :].rearrange("j p c -> p j c")
                        nc.sync.dma_start(dst, src)
                    pump(2)

                def a2a(b, hp):
                    nc.gpsimd.collective_compute(
                        "AllToAll", ALU.bypass,
                        replica_groups=[list(range(NCORES))],
                        ins=[bnc_in[b][hp].opt()],
                        outs=[bnc_out[b][hp].opt()],
                    )
                    pump(2)

                # ================= o_proj units ==========================
                att2_all = {}

                def oproj_units(b):
                    units = []

                    def load_att2():
                        att2 = []
                        wsel = []
                        for hp in range(NPAIR):
                            for i in range(NCORES):
                                t = a2p.tile([128, rows_h], BF16,
                                             tag=f"at2_{i}_{hp}",
                                             name=f"att2_{b}_{i}_{hp}")
                                eng = nc.sync if i % 2 == 0 else nc.gpsimd
                                eng.dma_start(t[:, :], bnc_out[b][hp][i, :, :])
                                att2.append(t)
                                wsel.append(i * NPAIR + hp)
                        att2_all[b] = (att2, wsel)
                    units.append(load_att2)

                    def chain_mm(oc, rt, ch0, tail_eng):
                        key = ("y", b, oc, rt)

                        def u():
                            att2, wsel = att2_all[b]
                            if ch0 == 0:
                                att2_all[key] = chp.tile(
                                    [rt_m, 512], FP32, tag="chain",
                                    name=f"y{b}_{oc}_{rt}")
                            yps = att2_all[key]
                            o0 = oc * 512
                            for ch in range(ch0, ch0 + 8):
                                nc.tensor.matmul(
                                    yps[:, :],
                                    att2[ch][:, rt * rt_m:(rt + 1) * rt_m],
                                    wot[wsel[ch]][:, o0:o0 + 512],
                                    start=(ch == 0), stop=(ch == NDC - 1))
                            if ch0 == 8:
                                ysb = yop.tile([rt_m, 512], FP32, tag="ysb",
                                               name=f"ysb{b}_{oc}_{rt}")
                                nc.scalar.copy(ysb[:, :], yps[:, :])
                                tail_eng.dma_start(
                                    out_h[b * rows_h + rt * rt_m:
                                          b * rows_h + (rt + 1) * rt_m,
                                          o0:o0 + 512],
                                    ysb[:, :])
                        return u

                    engs = [nc.sync, nc.gpsimd]
                    k = 0
                    for oc in range(D // 512):
                        for rt in range(nrt_h):
                            units.append(chain_mm(oc, rt, 0, None))
                            units.append(chain_mm(oc, rt, 8,
                                                  engs[k % len(engs)]))
                            k += 1
                    return units

                # ================= schedule ==============================
                # eager: b0 first two slices so attention(b0, qb0/qb1) has
                # its q/k/v
                for u in proj_slice_units(0, 0):
                    u()
                for u in proj_slice_units(0, 1):
                    u()
                # filler: rest of b0, then b1 loads+proj, wo, o_proj(b0)
                for un in load_xi_units(2):
                    fill.append(un)
                for un in proj_slice_units(1, 0):
                    fill.append(un)
                for un in proj_slice_units(1, 1):
                    fill.append(un)
                for un in proj_slice_units(2, 0):
                    fill.append(un)
                for un in load_xi_units(3):
                    fill.append(un)
                for un in proj_slice_units(2, 1):
                    fill.append(un)
                for un in proj_slice_units(3, 0):
                    fill.append(un)
                for un in proj_slice_units(3, 1):
                    fill.append(un)
                fill.append(release_wres)
                for un in wo_units():
                    fill.append(un)

                for hp in range(NPAIR):
                    for qb in range(nqb):
                        attn_block(0, hp, qb)
                    a2a(0, hp)
                # defer o_proj(b0): its att2 loads need the A2A(b0,hp1)
                # collective (~25us); pumping its chains too early stalls
                # the in-order PE and blocks attention(b1) behind them
                ou0 = oproj_units(0)
                schedule(ou0[:1], 12)
                schedule(ou0[1:], 20)
                for hp in range(NPAIR):
                    for qb in range(nqb):
                        attn_block(1, hp, qb)
                    a2a(1, hp)
                for un in oproj_units(1):
                    fill.append(un)
                while pending or fill:
                    pump(max(1, len(fill)))
                wo_state["ctx"].__exit__(None, None, None)

    nc.compile()
    return nc


# ===================== host side =====================

def _rope_tables(s):
    freqs = THETA ** (-np.arange(0, HD, 2, dtype=np.float64) / HD)   # [32]
    ang = np.arange(s, dtype=np.float64)[:, None] * freqs[None, :]   # [s, 32]
    cosT = np.cos(ang).T.astype(np.float32)                          # [32, s]
    sinT = np.sin(ang).T.astype(np.float32)
    c4 = np.tile(cosT, (4, 1))                                       # [128, s]
    s4 = np.tile(np.concatenate([-sinT, sinT], axis=0), (2, 1))      # [128, s]
    return (np.ascontiguousarray(c4).astype(NPBF16),
            np.ascontiguousarray(s4).astype(NPBF16))


def _mdiag():
    # keep[p, j] = 1 iff q-offset j >= k-offset p (diagonal 128-strip)
    u = np.arange(128)[None, :]
    p = np.arange(128)[:, None]
    return (u >= p).astype(np.float32)


def _perm_even_odd(w, n_heads_w):
    # reorder each head's 64 columns: even indices first, then odd
    perm = np.concatenate([np.arange(0, HD, 2), np.arange(1, HD, 2)])
    wr = w.reshape(D, n_heads_w, HD)[:, :, perm]
    return np.ascontiguousarray(wr.reshape(D, n_heads_w * HD))


def _is_causal(mask, s):
    m = np.asarray(mask, dtype=np.float32).reshape(s, s)
    tri = np.tril(np.ones((s, s), dtype=bool))
    return bool(np.all(m[tri] == 0.0) and np.all(m[~tri] <= -1e8))


def _bf16(a):
    return np.ascontiguousarray(np.asarray(a, np.float32).astype(NPBF16))


def make_in_maps(x, mask, wq, wk, wv, wo, s=S):
    """Shard full inputs into 8 per-core input dicts."""
    causal = _is_causal(mask, s)
    c4, s4 = _rope_tables(s)
    # fold the 1/sqrt(hd) score scale into wq on the host
    wq_p = _perm_even_odd(np.asarray(wq, np.float32) * 0.125, N_HEADS)
    wk_p = _perm_even_odd(np.asarray(wk, np.float32), N_KV)
    wv = np.asarray(wv, np.float32)
    wo_b = _bf16(wo)
    md = _mdiag().astype(NPBF16)
    xT = _bf16(np.asarray(x, np.float32).reshape(B * s, D).T)
    mT = None
    if not causal:
        mT = np.ascontiguousarray(np.asarray(mask, np.float32).reshape(s, s).T)

    in_maps = []
    for c in range(NCORES):
        wkv = np.concatenate([wk_p[:, c * HD:(c + 1) * HD],
                              wv[:, c * HD:(c + 1) * HD]], axis=1)
        im = {
            "xT": xT,
            "wq": _bf16(wq_p[:, c * QCOLS:(c + 1) * QCOLS]),
            "wkv": _bf16(wkv),
            "wo": wo_b,
            "c4": c4,
            "s4": s4,
        }
        if causal:
            im["mdiag"] = md
        else:
            im["maskT"] = mT
        in_maps.append(im)
    return causal, in_maps


def assemble_output(per_core_outs, s=S):
    rows_h = s // NCORES
    y = np.empty((B, s, D), dtype=np.float32)
    for c in range(NCORES):
        o = np.asarray(per_core_outs[c], np.float32)
        for b in range(B):
            y[b, c * rows_h:(c + 1) * rows_h, :] = \
                o[b * rows_h:(b + 1) * rows_h]
    return y


_GRAPH_CACHE = {}


def get_graph(causal, s=S):
    key = (causal, s)
    if key not in _GRAPH_CACHE:
        _GRAPH_CACHE[key] = build_graph(causal, s)
    return _GRAPH_CACHE[key]


def kernel(**inputs):
    x = np.asarray(inputs["x"], np.float32)
    mask = inputs["mask"]
    s = x.shape[1]
    causal, in_maps = make_in_maps(x, mask, inputs["wq"], inputs["wk"],
                                   inputs["wv"], inputs["wo"], s=s)
    nc = get_graph(causal, s)
    res = bass_utils.run_bass_kernel_spmd(nc, in_maps, core_ids=list(range(NCORES)))
    return assemble_output([res.results[c]["out"] for c in range(NCORES)], s=s)
